# revision 10
# baseline (speedup 1.0000x reference)
"""Trainium2 Bass kernel for ColorHistogramLoss.

Reference computation:
  brightness = mean(target, axis=1)           # [B,1,H,W]
  mask = brightness > 0.4
  soft 16-bin Gaussian histograms of pred/target per (b, c), masked,
  normalized; loss = mean |pred_hist - target_hist|.

Kernel strategy (8 NeuronCores, data-parallel over batch B=8), v2:
  Each core processes one image pair (pred[b], target[b]) [3,512,512].
  Channels are pair-stacked on the partition axis: a [128, 4096] tile
  holds channel A on partitions 0..63 and channel B on 64..127.

  Per bin k (center c_k = k/15) the Gaussian weight is
    w_k(x) = exp(-128 (x - c_k)^2).
  Instead of building 16 exp arguments on DVE + 16 ScalarE exps (the
  old ~248 us kernel), work is split three ways:

  * SEED bins (ACT): one ScalarE op per seed bin using the
    Derivative_Erf table: D_Erf(sqrt(128)*(xm - c_s)) =
    (2/sqrt(pi)) * w_s(xm), with a fused accum_out reduction.
  * CHAIN bins (DVE): w_{k+1} = w_k * G elementwise in bf16 (2x mode),
    where G = exp((256/15)(xm - 1/2)) is one ScalarE Exp per pair.
    The per-bin constant ratio is folded into a host-side unmix:
      hist_k = exp(8.5333 d - 128(c_k^2 - c_s^2)) * sum(W_k),  d = k-s.
  * Chain-bin reductions (PE): matmul with a 2-hot stationary
    ([128,32] bf16, ones on partitions 0..63 -> row 2j, 64..127 ->
    row 2j+1) accumulating all chain bins of a pair into one PSUM
    [32, 512] tile; a final DVE tensor_reduce collapses it to [32,1].

  Masking: xm = x - 100 on masked-out pixels (one f32 tensor_tensor per
  pair). Seeds evaluate to exactly 0 there and G evaluates to 0, so all
  chain bins inherit zero. The global (2/sqrt(pi)) amplitude cancels in
  the histogram normalization.

  ScalarE ops are ordered all-Exp-then-all-D_Erf so only two activation
  table loads happen per pass.

  Output per core: stats [128, 32] f32 — cols 0..17 per-partition seed
  accums (pair p, seed i at col 6p+i), cols 20..22 rows 0..31 the PE
  chain sums. The tiny normalize / L1 / mean finish runs on host.
"""

from contextlib import ExitStack

import numpy as np

import concourse.bass as bass
import concourse.tile as tile
from concourse import bacc, mybir
from concourse.bass_utils import run_bass_kernel_spmd

N_CORES = 8
C = 3
H = 512
W = 512
HW = H * W          # 262144
P = 128
HP = 64             # partitions per channel in a stacked pair
FP = HW // HP       # 4096
NB = 16
NPAIR = 3
F32 = mybir.dt.float32
BF16 = mybir.dt.bfloat16

SQ128 = float(np.sqrt(128.0))
GSC = 256.0 / 15.0          # 17.0667; G = exp(GSC*(xm-0.5))
MOFF = -100.0               # mask offset: xm = x - 100 where masked out

SEEDS = (0, 3, 6, 9, 12, 15)
CHAIN_BINS = tuple(k for k in range(NB) if k not in SEEDS)


def _seed_of(k):
    return max(s for s in SEEDS if s < k)


def _kernel_body(ctx, tc, stats_d, pred_d, target_d, repeat=1):
    nc = tc.nc
    spool = ctx.enter_context(tc.tile_pool(name="spool", bufs=1))
    xpool = ctx.enter_context(tc.tile_pool(name="xpool", bufs=3))
    mpool = ctx.enter_context(tc.tile_pool(name="mpool", bufs=1))
    xmpool = ctx.enter_context(tc.tile_pool(name="xmpool", bufs=1))
    gpool = ctx.enter_context(tc.tile_pool(name="gpool", bufs=1))
    sdpool = ctx.enter_context(tc.tile_pool(name="sdpool", bufs=7))
    wpool = ctx.enter_context(tc.tile_pool(name="wpool", bufs=3))
    ppool = ctx.enter_context(tc.tile_pool(name="ppool", bufs=1, space="PSUM"))

    # constants: ACT bias columns, 2-hot stationaries
    bias_g = spool.tile([P, 1], F32, tag="bias_g")
    nc.gpsimd.memset(bias_g[:], -0.5 * GSC)
    bias_s = spool.tile([P, len(SEEDS)], F32, tag="bias_s")
    for i, s in enumerate(SEEDS):
        nc.gpsimd.memset(bias_s[:, i : i + 1], -SQ128 * (s / 15.0))
    ones_t = spool.tile([P, 32 * len(CHAIN_BINS)], BF16, tag="ones")
    nc.gpsimd.memset(ones_t[:], 0.0)
    for j in range(len(CHAIN_BINS)):
        nc.gpsimd.memset(ones_t[:HP, 32 * j + 2 * j : 32 * j + 2 * j + 1], 1.0)
        nc.gpsimd.memset(ones_t[HP:, 32 * j + 2 * j + 1 : 32 * j + 2 * j + 2], 1.0)

    for _ in range(repeat):
        _emit_pass(
            tc, (spool, xpool, mpool, xmpool, gpool, sdpool, wpool, ppool),
            bias_g, bias_s, ones_t, stats_d, pred_d, target_d,
        )


def _emit_pass(tc, pools, bias_g, bias_s, ones_t, stats_d, pred_d, target_d):
    nc = tc.nc
    add = mybir.AluOpType.add
    mult = mybir.AluOpType.mult
    is_le = mybir.AluOpType.is_le
    spool, xpool, mpool, xmpool, gpool, sdpool, wpool, ppool = pools

    def chan_ap(dram, c):
        return dram[c].rearrange("(q g) -> q g", q=HP)

    # ---- loads -------------------------------------------------------
    # mask channels at base partition 0 first, then the pair stacks
    m0 = xpool.tile([P, FP], F32, tag="x")
    nc.sync.dma_start(out=m0[:HP, :], in_=chan_ap(target_d, 0))
    m2 = xpool.tile([P, FP], F32, tag="x")
    nc.sync.dma_start(out=m2[:HP, :], in_=chan_ap(target_d, 2))
    pair_srcs = [
        (chan_ap(target_d, 1), chan_ap(target_d, 2)),
        (chan_ap(pred_d, 2), chan_ap(target_d, 0)),
        (chan_ap(pred_d, 0), chan_ap(pred_d, 1)),
    ]
    pair_tiles = []
    for a_ap, b_ap in pair_srcs:
        t = xpool.tile([P, FP], F32, tag="x")
        nc.sync.dma_start(out=t[:HP, :], in_=a_ap)
        nc.sync.dma_start(out=t[HP:, :], in_=b_ap)
        pair_tiles.append(t)

    # ---- mask --------------------------------------------------------
    # moff = -100 where brightness sum <= 1.2 (masked out), else 0
    moff = mpool.tile([P, FP], F32, tag="moff")
    s = moff[:HP, :]
    nc.vector.tensor_tensor(out=s, in0=m0[:HP, :], in1=pair_tiles[0][:HP, :], op=add)
    nc.vector.tensor_tensor(out=s, in0=s, in1=m2[:HP, :], op=add)
    nc.vector.tensor_scalar(
        out=s, in0=s, scalar1=1.2, scalar2=MOFF, op0=is_le, op1=mult
    )
    nc.vector.tensor_scalar(
        out=moff[HP:, :], in0=s, scalar1=1.0, scalar2=None, op0=mult
    )

    stats_t = spool.tile([P, 32], F32)

    # ---- per-pair masked input, then G (all Exp ops emitted first) ---
    xm_tiles = []
    for p, x in enumerate(pair_tiles):
        xm = xmpool.tile([P, FP], BF16, tag=f"xm{p}")
        nc.vector.tensor_tensor(out=xm[:], in0=x[:], in1=moff[:], op=add)
        xm_tiles.append(xm)
    g_tiles = []
    for p in range(NPAIR):
        g = gpool.tile([P, FP], BF16, tag=f"g{p}")
        nc.scalar.activation(
            out=g[:], in_=xm_tiles[p][:],
            func=mybir.ActivationFunctionType.Exp,
            bias=bias_g[:, 0:1], scale=GSC,
        )
        g_tiles.append(g)

    # ---- seeds (D_Erf, fused accum), chains (DVE), reduces (PE) ------
    nchain = len(CHAIN_BINS)
    psums = []
    for p in range(NPAIR):
        ps = ppool.tile([P, 512], F32, tag=f"ps{p}")
        psums.append(ps)
    seed_tiles = [dict() for _ in range(NPAIR)]
    for p in range(NPAIR):
        for i, s_bin in enumerate(SEEDS):
            st = sdpool.tile([P, FP], BF16, tag="sd")
            nc.scalar.activation(
                out=st[:], in_=xm_tiles[p][:],
                func=mybir.ActivationFunctionType.Derivative_Erf,
                bias=bias_s[:, i : i + 1], scale=SQ128,
                accum_out=stats_t[:, 6 * p + i : 6 * p + i + 1],
            )
            seed_tiles[p][s_bin] = st

    for p in range(NPAIR):
        prev = dict(seed_tiles[p])
        for j, k in enumerate(CHAIN_BINS):
            wk = wpool.tile([P, FP], BF16, tag="w")
            nc.vector.tensor_tensor(
                out=wk[:], in0=prev[k - 1][:], in1=g_tiles[p][:], op=mult
            )
            prev[k] = wk
            for c in range(8):
                nc.tensor.matmul(
                    out=psums[p][0:32, :],
                    lhsT=ones_t[:, 32 * j : 32 * j + 32],
                    rhs=wk[:, 512 * c : 512 * (c + 1)],
                    start=(j == 0 and c == 0),
                    stop=(j == nchain - 1 and c == 7),
                )
        nc.vector.tensor_reduce(
            out=stats_t[0:32, 20 + p : 21 + p], in_=psums[p][0:32, :],
            axis=mybir.AxisListType.X, op=add,
        )

    nc.sync.dma_start(out=stats_d[:], in_=stats_t[:])


def build_nc(repeat=1):
    nc = bacc.Bacc(
        "TRN2", target_bir_lowering=False, debug=False, num_devices=N_CORES
    )
    pred = nc.dram_tensor("pred", [C, HW], F32, kind="ExternalInput").ap()
    target = nc.dram_tensor("target", [C, HW], F32, kind="ExternalInput").ap()
    stats = nc.dram_tensor("stats", [P, 32], F32, kind="ExternalOutput").ap()
    with tile.TileContext(nc) as tc:
        with ExitStack() as ctx:
            _kernel_body(ctx, tc, stats, pred, target, repeat=repeat)
    nc.compile()
    return nc


_NC_CACHE = {}


def _get_nc():
    if "nc" not in _NC_CACHE:
        _NC_CACHE["nc"] = build_nc()
    return _NC_CACHE["nc"]


def stats_to_hists(stats):
    """[128, 32] per-core stats -> hist [2, C, NB] (pred, target) f64."""
    stats = stats.astype(np.float64)
    cb = np.arange(NB) / 15.0
    # per (pair, half) raw histograms
    hp = np.zeros((NPAIR, 2, NB))
    for p in range(NPAIR):
        for i, s_bin in enumerate(SEEDS):
            col = stats[:, 6 * p + i]
            hp[p, 0, s_bin] = col[:HP].sum()
            hp[p, 1, s_bin] = col[HP:].sum()
        for j, k in enumerate(CHAIN_BINS):
            s_bin = _seed_of(k)
            d = k - s_bin
            sc = np.exp(8.533333333333333 * d - 128.0 * (cb[k] ** 2 - cb[s_bin] ** 2))
            hp[p, 0, k] = stats[2 * j, 20 + p] * sc
            hp[p, 1, k] = stats[2 * j + 1, 20 + p] * sc
    hist = np.empty((2, C, NB), np.float64)
    hist[1, 1] = hp[0, 0]  # target c1
    hist[1, 2] = hp[0, 1]  # target c2
    hist[0, 2] = hp[1, 0]  # pred c2
    hist[1, 0] = hp[1, 1]  # target c0
    hist[0, 0] = hp[2, 0]  # pred c0
    hist[0, 1] = hp[2, 1]  # pred c1
    return hist


def finish_on_host(stats_list):
    diffs = []
    for stats in stats_list:
        hist = stats_to_hists(stats)
        hist_n = hist / (hist.sum(axis=-1, keepdims=True) + 1e-7)
        diffs.append(np.abs(hist_n[0] - hist_n[1]))
    return np.array(np.mean(np.stack(diffs)), dtype=np.float32)


def run(pred, target, **spmd_kwargs):
    nc = _get_nc()
    pred = np.ascontiguousarray(np.asarray(pred, dtype=np.float32))
    target = np.ascontiguousarray(np.asarray(target, dtype=np.float32))
    assert pred.shape == (N_CORES, C, H, W), pred.shape
    in_maps = [
        {
            "pred": pred[b].reshape(C, HW),
            "target": target[b].reshape(C, HW),
        }
        for b in range(N_CORES)
    ]
    res = run_bass_kernel_spmd(nc, in_maps, core_ids=list(range(N_CORES)), **spmd_kwargs)
    loss = finish_on_host([res.results[b]["stats"] for b in range(N_CORES)])
    return loss, res


def kernel(pred, target):
    loss, _ = run(pred, target)
    return loss


# revision 14
# speedup vs baseline: 1.0144x; 1.0144x over previous
"""Trainium2 Bass kernel for ColorHistogramLoss.

Reference computation:
  brightness = mean(target, axis=1)           # [B,1,H,W]
  mask = brightness > 0.4
  soft 16-bin Gaussian histograms of pred/target per (b, c), masked,
  normalized; loss = mean |pred_hist - target_hist|.

Kernel strategy (8 NeuronCores, data-parallel over batch B=8), v2:
  Each core processes one image pair (pred[b], target[b]) [3,512,512].
  Channels are pair-stacked on the partition axis: a [128, 4096] tile
  holds channel A on partitions 0..63 and channel B on 64..127.

  Per bin k (center c_k = k/15) the Gaussian weight is
    w_k(x) = exp(-128 (x - c_k)^2).
  Instead of building 16 exp arguments on DVE + 16 ScalarE exps (the
  old ~248 us kernel), work is split three ways:

  * SEED bins (ACT): one ScalarE op per seed bin using the
    Derivative_Erf table: D_Erf(sqrt(128)*(xm - c_s)) =
    (2/sqrt(pi)) * w_s(xm), with a fused accum_out reduction.
  * CHAIN bins (DVE): w_{k+1} = w_k * G elementwise in bf16 (2x mode),
    where G = exp((256/15)(xm - 1/2)) is one ScalarE Exp per pair.
    The per-bin constant ratio is folded into a host-side unmix:
      hist_k = exp(8.5333 d - 128(c_k^2 - c_s^2)) * sum(W_k),  d = k-s.
  * Chain-bin reductions (PE): matmul with a 2-hot stationary
    ([128,32] bf16, ones on partitions 0..63 -> row 2j, 64..127 ->
    row 2j+1) accumulating all chain bins of a pair into one PSUM
    [32, 512] tile; a final DVE tensor_reduce collapses it to [32,1].

  Masking: xm = x - 100 on masked-out pixels (one f32 tensor_tensor per
  pair). Seeds evaluate to exactly 0 there and G evaluates to 0, so all
  chain bins inherit zero. The global (2/sqrt(pi)) amplitude cancels in
  the histogram normalization.

  ScalarE ops are ordered all-Exp-then-all-D_Erf so only two activation
  table loads happen per pass.

  Output per core: stats [128, 32] f32 — cols 0..17 per-partition seed
  accums (pair p, seed i at col 6p+i), cols 20..22 rows 0..31 the PE
  chain sums. The tiny normalize / L1 / mean finish runs on host.
"""

from contextlib import ExitStack

import numpy as np

import concourse.bass as bass
import concourse.tile as tile
from concourse import bacc, mybir
from concourse.bass_utils import run_bass_kernel_spmd

N_CORES = 8
C = 3
H = 512
W = 512
HW = H * W          # 262144
P = 128
HP = 64             # partitions per channel in a stacked pair
FP = HW // HP       # 4096
NB = 16
NPAIR = 3
F32 = mybir.dt.float32
BF16 = mybir.dt.bfloat16

SQ128 = float(np.sqrt(128.0))
GSC = 256.0 / 15.0          # 17.0667; G = exp(GSC*(xm-0.5))
MOFF = -100.0               # mask offset: xm = x - 100 where masked out

SEEDS = (0, 3, 6, 9, 12, 15)
CHAIN_BINS = tuple(k for k in range(NB) if k not in SEEDS)

# offload toggles: run the masked-input build / mask arithmetic on the
# (otherwise idle) GPSIMD engine instead of DVE.  GPSIMD shares SBUF
# ports with DVE, so these are empirical knobs.
XM_ON_GPS = True
MASK_ON_GPS = False


def _seed_of(k):
    return max(s for s in SEEDS if s < k)


def _kernel_body(ctx, tc, stats_d, pred_d, target_d, repeat=1):
    nc = tc.nc
    spool = ctx.enter_context(tc.tile_pool(name="spool", bufs=1))
    xpool = ctx.enter_context(tc.tile_pool(name="xpool", bufs=3))
    mpool = ctx.enter_context(tc.tile_pool(name="mpool", bufs=1))
    xmpool = ctx.enter_context(tc.tile_pool(name="xmpool", bufs=1))
    gpool = ctx.enter_context(tc.tile_pool(name="gpool", bufs=1))
    sdpool = ctx.enter_context(tc.tile_pool(name="sdpool", bufs=7))
    wpool = ctx.enter_context(tc.tile_pool(name="wpool", bufs=3))
    ppool = ctx.enter_context(tc.tile_pool(name="ppool", bufs=1, space="PSUM"))

    # constants: ACT bias columns, 2-hot stationaries
    bias_g = spool.tile([P, 1], F32, tag="bias_g")
    nc.gpsimd.memset(bias_g[:], -0.5 * GSC)
    bias_s = spool.tile([P, len(SEEDS)], F32, tag="bias_s")
    for i, s in enumerate(SEEDS):
        nc.gpsimd.memset(bias_s[:, i : i + 1], -SQ128 * (s / 15.0))
    ones_t = spool.tile([P, 32 * len(CHAIN_BINS)], BF16, tag="ones")
    nc.gpsimd.memset(ones_t[:], 0.0)
    for j in range(len(CHAIN_BINS)):
        nc.gpsimd.memset(ones_t[:HP, 32 * j + 2 * j : 32 * j + 2 * j + 1], 1.0)
        nc.gpsimd.memset(ones_t[HP:, 32 * j + 2 * j + 1 : 32 * j + 2 * j + 2], 1.0)

    for _ in range(repeat):
        _emit_pass(
            tc, (spool, xpool, mpool, xmpool, gpool, sdpool, wpool, ppool),
            bias_g, bias_s, ones_t, stats_d, pred_d, target_d,
        )


def _emit_pass(tc, pools, bias_g, bias_s, ones_t, stats_d, pred_d, target_d):
    nc = tc.nc
    add = mybir.AluOpType.add
    mult = mybir.AluOpType.mult
    is_le = mybir.AluOpType.is_le
    spool, xpool, mpool, xmpool, gpool, sdpool, wpool, ppool = pools

    def chan_ap(dram, c):
        return dram[c].rearrange("(q g) -> q g", q=HP)

    # ---- loads -------------------------------------------------------
    # mask channels at base partition 0 first, then the pair stacks
    m0 = xpool.tile([P, FP], F32, tag="x")
    nc.sync.dma_start(out=m0[:HP, :], in_=chan_ap(target_d, 0))
    m2 = xpool.tile([P, FP], F32, tag="x")
    nc.sync.dma_start(out=m2[:HP, :], in_=chan_ap(target_d, 2))
    pair_srcs = [
        (chan_ap(target_d, 1), chan_ap(target_d, 2)),
        (chan_ap(pred_d, 2), chan_ap(target_d, 0)),
        (chan_ap(pred_d, 0), chan_ap(pred_d, 1)),
    ]
    pair_tiles = []
    for a_ap, b_ap in pair_srcs:
        t = xpool.tile([P, FP], F32, tag="x")
        nc.sync.dma_start(out=t[:HP, :], in_=a_ap)
        nc.sync.dma_start(out=t[HP:, :], in_=b_ap)
        pair_tiles.append(t)

    # ---- mask --------------------------------------------------------
    # moff = -100 where brightness sum <= 1.2 (masked out), else 0
    moff = mpool.tile([P, FP], F32, tag="moff")
    s = moff[:HP, :]
    meng = nc.gpsimd if MASK_ON_GPS else nc.vector
    meng.tensor_tensor(out=s, in0=m0[:HP, :], in1=pair_tiles[0][:HP, :], op=add)
    meng.tensor_tensor(out=s, in0=s, in1=m2[:HP, :], op=add)
    meng.tensor_scalar(
        out=s, in0=s, scalar1=1.2, scalar2=MOFF, op0=is_le, op1=mult
    )
    nc.vector.tensor_scalar(
        out=moff[HP:, :], in0=s, scalar1=1.0, scalar2=None, op0=mult
    )

    stats_t = spool.tile([P, 32], F32)

    # ---- per-pair masked input, then G (all Exp ops emitted first) ---
    xm_tiles = []
    xeng = nc.gpsimd if XM_ON_GPS else nc.vector
    for p, x in enumerate(pair_tiles):
        xm = xmpool.tile([P, FP], BF16, tag=f"xm{p}")
        xeng.tensor_tensor(out=xm[:], in0=x[:], in1=moff[:], op=add)
        xm_tiles.append(xm)
    g_tiles = []
    for p in range(NPAIR):
        g = gpool.tile([P, FP], BF16, tag=f"g{p}")
        nc.scalar.activation(
            out=g[:], in_=xm_tiles[p][:],
            func=mybir.ActivationFunctionType.Exp,
            bias=bias_g[:, 0:1], scale=GSC,
        )
        g_tiles.append(g)

    # ---- seeds (D_Erf, fused accum), chains (DVE), reduces (PE) ------
    nchain = len(CHAIN_BINS)
    psums = []
    for p in range(NPAIR):
        ps = ppool.tile([P, 512], F32, tag=f"ps{p}")
        psums.append(ps)
    seed_tiles = [dict() for _ in range(NPAIR)]
    for p in range(NPAIR):
        for i, s_bin in enumerate(SEEDS):
            st = sdpool.tile([P, FP], BF16, tag="sd")
            nc.scalar.activation(
                out=st[:], in_=xm_tiles[p][:],
                func=mybir.ActivationFunctionType.Derivative_Erf,
                bias=bias_s[:, i : i + 1], scale=SQ128,
                accum_out=stats_t[:, 6 * p + i : 6 * p + i + 1],
            )
            seed_tiles[p][s_bin] = st

    for p in range(NPAIR):
        prev = dict(seed_tiles[p])
        for j, k in enumerate(CHAIN_BINS):
            wk = wpool.tile([P, FP], BF16, tag="w")
            nc.vector.tensor_tensor(
                out=wk[:], in0=prev[k - 1][:], in1=g_tiles[p][:], op=mult
            )
            prev[k] = wk
            for c in range(8):
                nc.tensor.matmul(
                    out=psums[p][0:32, :],
                    lhsT=ones_t[:, 32 * j : 32 * j + 32],
                    rhs=wk[:, 512 * c : 512 * (c + 1)],
                    start=(j == 0 and c == 0),
                    stop=(j == nchain - 1 and c == 7),
                )
        nc.vector.tensor_reduce(
            out=stats_t[0:32, 20 + p : 21 + p], in_=psums[p][0:32, :],
            axis=mybir.AxisListType.X, op=add,
        )

    nc.sync.dma_start(out=stats_d[:], in_=stats_t[:])


def build_nc(repeat=1):
    nc = bacc.Bacc(
        "TRN2", target_bir_lowering=False, debug=False, num_devices=N_CORES
    )
    pred = nc.dram_tensor("pred", [C, HW], F32, kind="ExternalInput").ap()
    target = nc.dram_tensor("target", [C, HW], F32, kind="ExternalInput").ap()
    stats = nc.dram_tensor("stats", [P, 32], F32, kind="ExternalOutput").ap()
    with tile.TileContext(nc) as tc:
        with ExitStack() as ctx:
            _kernel_body(ctx, tc, stats, pred, target, repeat=repeat)
    nc.compile()
    return nc


_NC_CACHE = {}


def _get_nc():
    if "nc" not in _NC_CACHE:
        _NC_CACHE["nc"] = build_nc()
    return _NC_CACHE["nc"]


def stats_to_hists(stats):
    """[128, 32] per-core stats -> hist [2, C, NB] (pred, target) f64."""
    stats = stats.astype(np.float64)
    cb = np.arange(NB) / 15.0
    # per (pair, half) raw histograms
    hp = np.zeros((NPAIR, 2, NB))
    for p in range(NPAIR):
        for i, s_bin in enumerate(SEEDS):
            col = stats[:, 6 * p + i]
            hp[p, 0, s_bin] = col[:HP].sum()
            hp[p, 1, s_bin] = col[HP:].sum()
        for j, k in enumerate(CHAIN_BINS):
            s_bin = _seed_of(k)
            d = k - s_bin
            sc = np.exp(8.533333333333333 * d - 128.0 * (cb[k] ** 2 - cb[s_bin] ** 2))
            hp[p, 0, k] = stats[2 * j, 20 + p] * sc
            hp[p, 1, k] = stats[2 * j + 1, 20 + p] * sc
    hist = np.empty((2, C, NB), np.float64)
    hist[1, 1] = hp[0, 0]  # target c1
    hist[1, 2] = hp[0, 1]  # target c2
    hist[0, 2] = hp[1, 0]  # pred c2
    hist[1, 0] = hp[1, 1]  # target c0
    hist[0, 0] = hp[2, 0]  # pred c0
    hist[0, 1] = hp[2, 1]  # pred c1
    return hist


def finish_on_host(stats_list):
    diffs = []
    for stats in stats_list:
        hist = stats_to_hists(stats)
        hist_n = hist / (hist.sum(axis=-1, keepdims=True) + 1e-7)
        diffs.append(np.abs(hist_n[0] - hist_n[1]))
    return np.array(np.mean(np.stack(diffs)), dtype=np.float32)


def run(pred, target, **spmd_kwargs):
    nc = _get_nc()
    pred = np.ascontiguousarray(np.asarray(pred, dtype=np.float32))
    target = np.ascontiguousarray(np.asarray(target, dtype=np.float32))
    assert pred.shape == (N_CORES, C, H, W), pred.shape
    in_maps = [
        {
            "pred": pred[b].reshape(C, HW),
            "target": target[b].reshape(C, HW),
        }
        for b in range(N_CORES)
    ]
    res = run_bass_kernel_spmd(nc, in_maps, core_ids=list(range(N_CORES)), **spmd_kwargs)
    loss = finish_on_host([res.results[b]["stats"] for b in range(N_CORES)])
    return loss, res


def kernel(pred, target):
    loss, _ = run(pred, target)
    return loss
